# revision 48
# baseline (speedup 1.0000x reference)
"""Multi-head self-attention (B=2, S=2048, D=1024, H=16) on 8 trn2 cores.

Sharding: core c = b*4 + g  (b = batch, g = head-group of 4 heads).
Each core computes, for its batch b and heads 4g..4g+3:
  Qt = (Wq_g^T x_b^T + bq_g),  Kt likewise   -> [256, 2048] feature-major
  V  = x_b Wv_g                               -> [2048, 256] seq-major (no bias)
  scoresT[k,q] = sum_d Kt[d,k] Qt[d,q]        (per 128-key chunk)
  expT = exp(0.125 * scoresT)                 (ACT, straight from PSUM)
  ctxT/rowsum via PV matmul with V||ones      -> psum [65, 512]
  ctxT normalized by 1/rowsum (DMA partition-broadcast + DVE mult)
  y_partial = ctx_g @ Wo_g                    -> [2048, 1024]
Host: Y[b] = sum_g y_partial + (bo + bv @ Wo).
"""

import sys

sys.path.insert(0, "/opt/trn_rl_repo")

import numpy as np

import concourse.bass as bass
import concourse.mybir as mybir
import concourse.tile as tile

F32 = mybir.dt.float32
F32R = mybir.dt.float32r
BF16 = mybir.dt.bfloat16
F16 = mybir.dt.float16
MMDT = F16                     # dtype for all matmul operands (f16: 1 cyc/row like bf16, 10-bit mantissa)
AF = mybir.ActivationFunctionType

D = 1024          # d_model
S = 2048          # sequence length
HPC = 4           # heads per core
DK = 64           # head dim
E = HPC * DK      # 256 features per core
N_CORES = 8





_ENGINE_OPS = {
    "InstMatmult", "InstActivation", "InstTensorCopy", "InstTensorTensor",
    "InstReciprocal", "InstTensorReduce", "InstMemset", "InstIota",
    "InstTensorScalarPtr", "InstTranspose", "InstLdweights",
    "InstDMACopy", "InstDrain", "InstNoOp",
}


def _legalize_matmul_waits(nc):
    """walrus allows at most 1 sync wait on engine compute instructions; Tile
    sometimes emits more. Move the excess onto EventSemaphore instructions
    (cap 2 each) placed immediately before in same-engine program order."""
    for f in nc.m.functions:
        for bb in f.blocks:
            out = []
            changed = False
            for i in bb.instructions:
                si = getattr(i, "sync_info", None)
                if (
                    type(i).__name__ in _ENGINE_OPS
                    and si is not None
                    and si.on_wait
                    and len(si.on_wait) > 1
                ):
                    waits = list(si.on_wait)
                    excess, keep = waits[:-1], waits[-1:]
                    for c in range(0, len(excess), 2):
                        ev = mybir.InstEventSemaphore(
                            name=f"{i.name}-mmw{c}", ins=[], outs=[]
                        )
                        ev.engine = i.engine
                        ev.sync_info = mybir.SyncInfo(
                            on_wait=excess[c:c + 2], on_update=[]
                        )
                        out.append(ev)
                    i.sync_info = mybir.SyncInfo(
                        on_wait=keep, on_update=list(si.on_update)
                    )
                    changed = True
                out.append(i)
            if changed:
                bb.instructions = out


def build_nc():
    nc = bass.Bass()

    xt = nc.dram_tensor("xt", [D, S], MMDT, kind="ExternalInput")
    wq = nc.dram_tensor("wq", [D, E], MMDT, kind="ExternalInput")
    wk = nc.dram_tensor("wk", [D, E], MMDT, kind="ExternalInput")
    wv = nc.dram_tensor("wv", [D, E], MMDT, kind="ExternalInput")
    wo = nc.dram_tensor("wo", [E, D], MMDT, kind="ExternalInput")
    bq = nc.dram_tensor("bq", [E], F32, kind="ExternalInput")
    bk = nc.dram_tensor("bk", [E], F32, kind="ExternalInput")
    y = nc.dram_tensor("y", [S, D], F32, kind="ExternalOutput")

    KT = D // 128     # 8 k-tiles over d_model
    QC = S // 512     # 4 q-chunks of 512
    SC = S // 128     # 16 seq chunks of 128 (key chunks)
    ET = E // 128     # 2 feature tiles

    CB = S // DK      # 32 key blocks of 64

    with tile.TileContext(nc) as tc:
        with tc.tile_pool(name="persist", bufs=1) as pp:
            # ---- persistent tiles ----
            # Attention runs on head PAIRS (A = head 2t at partitions 0:64,
            # B = head 2t+1 at 64:128). Kt and V are stored block-diagonally
            # per 64-wide key block c so every attention matmul contracts
            # over the full 128 partitions (keeps the PE HAM clock warm):
            #   ktbd[t][:, c, :] = [[Kt_A(c), 0], [0, Kt_B(c)]]
            #   vbd [t][:, c, :] = [[V_A(c), 0], [0, V_B(c)]]
            qt_sb = [pp.tile([128, S], MMDT, tag=f"qt{t}", name=f"qt{t}") for t in range(ET)]
            ktbd = [pp.tile([128, CB, 128], MMDT, tag=f"ktbd{t}", name=f"ktbd{t}") for t in range(ET)]
            # vdup[c]: V for key block c (+ ones column), duplicated on both
            # partition halves so either exp row range can contract with it.
            vdup = [pp.tile([128, HPC, DK + 1], MMDT, tag=f"vd{c}", name=f"vd{c}")
                    for c in range(CB)]
            ctx_sb = [pp.tile([128, S], MMDT, tag=f"ctx{t}", name=f"ctx{t}") for t in range(ET)]
            wo_sb = [pp.tile([128, D], MMDT, tag=f"wo{t}", name=f"wo{t}") for t in range(ET)]
            bq_sb = pp.tile([128, ET], F32, tag="bq")
            bk_sb = pp.tile([128, ET], F32, tag="bk")

            nc.sync.dma_start(bq_sb, bq.rearrange("(t p) -> p t", p=128))
            nc.sync.dma_start(bk_sb, bk.rearrange("(t p) -> p t", p=128))
            for t in range(ET):
                nc.sync.dma_start(wo_sb[t], wo[t * 128:(t + 1) * 128, :])

            ones_sb = pp.tile([128, HPC], F32, tag="ones")
            zeros_sb = pp.tile([128, 2048], F32, tag="zeros")
            nc.vector.memset(ones_sb, 1.0)
            nc.vector.memset(zeros_sb, 0.0)
            # ones columns of vdup; zero the off-diagonal blocks of ktbd
            for c in range(CB):
                nc.vector.tensor_copy(vdup[c][:, :, DK:DK + 1],
                                      ones_sb[:, :, None])
            zblk = zeros_sb.rearrange("p (c k) -> p c k", k=64)
            for t in range(ET):
                nc.vector.tensor_copy(ktbd[t][0:64, :, 64:128], zblk[0:64])
                nc.vector.tensor_copy(ktbd[t][64:128, :, 0:64], zblk[64:128])

            # ---- stage A: projections ----
            with (
                tc.tile_pool(name="stageA", bufs=1) as pa,
                tc.tile_pool(name="psA", bufs=8, space="PSUM") as psA,
            ):
                xt_sb = [pa.tile([128, S], MMDT, tag=f"xt{k}", name=f"xt{k}") for k in range(KT)]
                wq_sb = pa.tile([128, KT, E], MMDT, tag="wq")
                wk_sb = pa.tile([128, KT, E], MMDT, tag="wk")
                wv_sb = pa.tile([128, KT, E], MMDT, tag="wv")
                for k in range(KT):
                    nc.sync.dma_start(xt_sb[k], xt[k * 128:(k + 1) * 128, :])
                    nc.sync.dma_start(wq_sb[:, k, :], wq[k * 128:(k + 1) * 128, :])
                    nc.sync.dma_start(wk_sb[:, k, :], wk[k * 128:(k + 1) * 128, :])
                    nc.sync.dma_start(wv_sb[:, k, :], wv[k * 128:(k + 1) * 128, :])

                # Qt/Kt: feature-major [e, s];  out = W_tile^T @ xt.
                # k-outer with 8 resident psum accumulators so the first
                # matmuls start as soon as xt[0]/w[0] land.
                def emit_proj(wi):
                    w_sb, b_sb = ((wq_sb, bq_sb), (wk_sb, bk_sb))[wi]
                    pss = [
                        psA.tile([128, 512], F32, tag="proj",
                                 name=f"pp{wi}_{t}_{qc}")
                        for t in range(ET) for qc in range(QC)
                    ]
                    for k in range(KT):
                        for t in range(ET):
                            for qc in range(QC):
                                nc.tensor.matmul(
                                    pss[t * QC + qc],
                                    w_sb[:, k, t * 128:(t + 1) * 128],
                                    xt_sb[k][:, qc * 512:(qc + 1) * 512],
                                    start=(k == 0),
                                    stop=(k == KT - 1),
                                )
                    for t in range(ET):
                        for qc in range(QC):
                            ps = pss[t * QC + qc]
                            if wi == 0:
                                nc.scalar.activation(
                                    qt_sb[t][:, qc * 512:(qc + 1) * 512],
                                    ps,
                                    AF.Identity, bias=b_sb[:, t:t + 1],
                                )
                            else:
                                # Kt: scatter each half into its diagonal
                                # block of ktbd (c-blocks qc*8 .. qc*8+8)
                                c0 = qc * 8
                                nc.scalar.activation(
                                    ktbd[t][0:64, c0:c0 + 8, 0:64],
                                    ps[0:64].rearrange(
                                        "p (c k) -> p c k", k=64),
                                    AF.Identity, bias=b_sb[0:64, t:t + 1],
                                )
                                nc.scalar.activation(
                                    ktbd[t][64:128, c0:c0 + 8, 64:128],
                                    ps[64:128].rearrange(
                                        "p (c k) -> p c k", k=64),
                                    AF.Identity, bias=b_sb[64:128, t:t + 1],
                                )

                # V: seq-major [s, e];  out = xt_tile^T @ wv. Evict to an
                # aligned staging tile, then DMA-duplicate each 64-row key
                # block onto both partition halves of vdup (DMA may cross
                # partitions; compute engines may not).
                def emit_v(s_range):
                    for s in s_range:
                        ps = psA.tile([128, E], F32, tag="proj",
                                      name=f"vps{s}")
                        for k in range(KT):
                            nc.tensor.matmul(
                                ps,
                                xt_sb[k][:, s * 128:(s + 1) * 128],
                                wv_sb[:, k, :],
                                start=(k == 0),
                                stop=(k == KT - 1),
                            )
                        vs = pp.tile([128, HPC, DK], MMDT, tag="vstage",
                                     name=f"vstage{s}", bufs=4)
                        nc.vector.tensor_copy(
                            vs, ps.rearrange("p (h d) -> p h d", d=DK))
                        for par in range(2):
                            src = vs[par * 64:(par + 1) * 64]
                            c = 2 * s + par
                            nc.sync.dma_start(vdup[c][0:64, :, 0:DK], src)
                            nc.sync.dma_start(vdup[c][64:128, :, 0:DK], src)

                # order: Qt, all V, then Kt — attention scores depend on
                # Qt+Kt, so Kt last puts attention right behind it in the
                # PE FIFO with V already done.
                emit_proj(0)
                emit_v(range(0, SC))
                emit_proj(1)

            # ---- stage B: attention ----
            with (
                tc.tile_pool(name="stageB", bufs=3) as pb,
                tc.tile_pool(name="dramB", bufs=3, space="DRAM") as dramB,
                tc.tile_pool(name="psS", bufs=2, space="PSUM") as psS,
                tc.tile_pool(name="psC", bufs=4, space="PSUM") as psC,
            ):
                # Scores are block-diagonal (K=128: keeps the PE HAM clock
                # warm; head A lands on psum rows 0:64, head B on 64:128).
                # One exp call per key block covers both heads. PV runs as
                # K=64 row-tile pairs (A rows 0:64 || B rows 64:128 execute
                # concurrently) against vdup, with the ones column giving
                # the softmax denominator as psum row 64.
                def emit_scores(t, qh, c):
                    sc_ps = psS.tile([128, 1024], F32, tag="sc",
                                     name=f"sc{t}_{qh}_{c}")
                    for j in range(2):
                        nc.tensor.matmul(
                            sc_ps[:, j * 512:(j + 1) * 512],
                            ktbd[t][:, c, :],
                            qt_sb[t][:, qh * 1024 + j * 512:
                                     qh * 1024 + (j + 1) * 512],
                            start=True, stop=True,
                        )
                    return sc_ps

                for t in range(ET):                   # head pair (2t, 2t+1)
                    for qh in range(2):               # q halves of 1024
                        ctx_ps = {
                            (hp, j): psC.tile([DK + 1, 512], F32, tag="ctx",
                                              name=f"ctxps{t}_{qh}_{hp}_{j}")
                            for hp in range(2) for j in range(2)
                        }
                        sc_ps = emit_scores(t, qh, 0)
                        for c in range(CB):
                            # software pipeline: next block's scores go to
                            # the PE queue before this block's PV so PE isn't
                            # head-of-line blocked on the exp result.
                            sc_next = (emit_scores(t, qh, c + 1)
                                       if c + 1 < CB else None)
                            ex = pb.tile([128, 1024], MMDT, tag="ex",
                                         name=f"ex{t}_{qh}_{c}", bufs=4)
                            nc.scalar.activation(ex, sc_ps, AF.Exp,
                                                 scale=0.125)
                            for j in range(2):
                                for hp in range(2):
                                    base = hp * 64
                                    nc.tensor.matmul(
                                        ctx_ps[hp, j],
                                        vdup[c][base:base + 64, 2 * t + hp, :],
                                        ex[base:base + 64,
                                           j * 512:(j + 1) * 512],
                                        start=(c == 0),
                                        stop=(c == CB - 1),
                                    )
                            sc_ps = sc_next
                        # Evict psums to SBUF right away (frees the ctx banks
                        # before the slow reciprocals hit the DVE queue),
                        # then normalize from staging off the critical path.
                        stgs = []
                        for hp in range(2):
                            stg = pb.tile([DK + 1, 1024], F32, tag=f"stg{hp}",
                                          name=f"stg{t}_{qh}_{hp}")
                            for j in range(2):
                                nc.vector.tensor_copy(
                                    stg[:, j * 512:(j + 1) * 512],
                                    ctx_ps[hp, j],
                                )
                            stgs.append(stg)
                        for hp in range(2):
                            stg = stgs[hp]
                            # reciprocal of the rowsum on 64 partitions
                            # ([64, 16] via DRAM scatter) — ~40x faster than
                            # on the natural [1, 1024] single-partition row.
                            rs_dr = dramB.tile(
                                [1, 1024], F32, tag=f"rs_dr{hp}",
                                name=f"rsdr{t}_{qh}_{hp}",
                            )
                            nc.sync.dma_start(rs_dr, stg[DK:DK + 1, :])
                            rs64 = pb.tile([64, 16], F32, tag=f"rs64{hp}",
                                           name=f"rs64{t}_{qh}_{hp}")
                            nc.sync.dma_start(
                                rs64, rs_dr.rearrange("o (p f) -> (o p) f", f=16)
                            )
                            rc64 = pb.tile([64, 16], F32, tag=f"rc64{hp}",
                                           name=f"rc64{t}_{qh}_{hp}")
                            nc.vector.reciprocal(rc64, rs64)
                            rc_dr = dramB.tile(
                                [1, 1024], F32, tag=f"rc_dr{hp}",
                                name=f"rcdr{t}_{qh}_{hp}",
                            )
                            nc.sync.dma_start(
                                rc_dr.rearrange("o (p f) -> (o p) f", f=16), rc64
                            )
                            rb = pb.tile([64, 1024], F32, tag=f"rb{hp}",
                                         name=f"rb{t}_{qh}_{hp}")
                            nc.sync.dma_start(
                                rb, rc_dr.to_broadcast([64, 1024])
                            )
                            nc.vector.tensor_mul(
                                ctx_sb[t][hp * 64:hp * 64 + 64,
                                          qh * 1024:(qh + 1) * 1024],
                                stg[0:DK, :],
                                rb,
                            )

            # ---- stage C: output projection ----
            with (
                tc.tile_pool(name="stageC", bufs=6) as pc,
                tc.tile_pool(name="psY", bufs=6, space="PSUM") as psY,
            ):
                for qt in range(SC):
                    ys = pc.tile([128, 1024], F32, tag="ys",
                                 name=f"ys{qt}")
                    for n in range(2):
                        yp = psY.tile([128, 512], F32, tag="y",
                                      name=f"yp{qt}_{n}")
                        for t in range(ET):
                            nc.tensor.matmul(
                                yp,
                                ctx_sb[t][:, qt * 128:(qt + 1) * 128],
                                wo_sb[t][:, n * 512:(n + 1) * 512],
                                start=(t == 0),
                                stop=(t == ET - 1),
                            )
                        # alternate eviction engine so DVE isn't the Y serial
                        # bottleneck
                        if n == 0:
                            nc.vector.tensor_copy(
                                ys[:, n * 512:(n + 1) * 512], yp)
                        else:
                            nc.scalar.copy(ys[:, n * 512:(n + 1) * 512], yp)
                    nc.sync.dma_start(y[qt * 128:(qt + 1) * 128, :], ys)
    _legalize_matmul_waits(nc)
    return nc


_NC_CACHE = None


def _get_nc():
    global _NC_CACHE
    if _NC_CACHE is None:
        _NC_CACHE = build_nc()
    return _NC_CACHE


def make_in_maps(inputs):
    mmnp = mybir.dt.np(MMDT)
    x = np.asarray(inputs["x"], dtype=np.float32)
    Wq = np.asarray(inputs["Wq"], dtype=np.float32)
    Wk = np.asarray(inputs["Wk"], dtype=np.float32)
    Wv = np.asarray(inputs["Wv"], dtype=np.float32)
    Wo = np.asarray(inputs["Wo"], dtype=np.float32)
    bq = np.asarray(inputs["bq"], dtype=np.float32)
    bk = np.asarray(inputs["bk"], dtype=np.float32)

    in_maps = []
    for c in range(N_CORES):
        b, g = c // 4, c % 4
        sl = slice(g * E, (g + 1) * E)
        in_maps.append({
            "xt": np.ascontiguousarray(x[b].T).astype(mmnp),
            "wq": np.ascontiguousarray(Wq[:, sl]).astype(mmnp),
            "wk": np.ascontiguousarray(Wk[:, sl]).astype(mmnp),
            "wv": np.ascontiguousarray(Wv[:, sl]).astype(mmnp),
            "wo": np.ascontiguousarray(Wo[sl, :]).astype(mmnp),
            "bq": np.ascontiguousarray(bq[sl]),
            "bk": np.ascontiguousarray(bk[sl]),
        })
    return in_maps


def kernel(x, Wq, bq, Wk, bk, Wv, bv, Wo, bo):
    from concourse.bass_utils import run_bass_kernel_spmd

    x = np.asarray(x, dtype=np.float32)
    Wv = np.asarray(Wv, dtype=np.float32)
    Wo = np.asarray(Wo, dtype=np.float32)
    bv = np.asarray(bv, dtype=np.float32)
    bo = np.asarray(bo, dtype=np.float32)

    B = x.shape[0]
    nc = _get_nc()
    in_maps = make_in_maps({
        "x": x, "Wq": Wq, "Wk": Wk, "Wv": Wv, "Wo": Wo, "bq": bq, "bk": bk,
    })

    res = run_bass_kernel_spmd(nc, in_maps, core_ids=list(range(N_CORES)))

    bias_total = bo + bv @ Wo  # [D]
    out = np.zeros((B, S, D), dtype=np.float32)
    for c in range(N_CORES):
        out[c // 4] += res.results[c]["y"]
    out += bias_total[None, None, :]
    return out
